# revision 5
# baseline (speedup 1.0000x reference)
"""Trainium2 Bass kernel for nn_DRA_40072044872030.

Key mathematical identity: in the reference, `_attention_module` applies
softmax over an axis of size 1, which is identically 1.0, so the module is
an exact identity map (wp = p * 1.0). The network therefore reduces to
`_composite_head(feature, ref_feature, ...)`:

    d = ref_feature - feature                         [B, 200, 56, 56]
    h = relu(BN(conv3x3(d, W) + cb))                  [B, 200, 56, 56]
    s = |conv1x1(h, w_s) + sb|                        [B, 56*56]
    out[b] = mean(top_313(s[b]))                      [B, 1]

Device implementation (8 NeuronCores, batch-sharded 2 images/core):
  - BN folded into conv weights/bias on host (weight preprocessing).
  - Images shipped in a zero-padded flat layout [margin | 58*58 | margin]
    per channel so the 3x3 conv becomes 9 shifted contiguous matmuls
    accumulated in PSUM; contraction ci -> 2x100 groups, outputs co ->
    2x100 groups.  d = ref - feat computed on device (in-place DVE sub).
  - float32r (full-rate fp32 matmul mode, ~12-bit mantissa) by default;
    exact fp32 matmul mode via PRECISION = "f32" (4x slower PE).
  - Exact top-k mean via GPSIMD kth_largest (exact 313th-largest value t),
    then mean = (sum(s where s > t) + (313 - count(s > t)) * t) / 313.
"""

import sys

if "/opt/trn_rl_repo" not in sys.path:
    sys.path.insert(0, "/opt/trn_rl_repo")

import numpy as np

import concourse.bass as bass
import concourse.tile as tile
from concourse import bacc, bass_isa, mybir
from concourse.bass_utils import run_bass_kernel_spmd

F32 = mybir.dt.float32
F32R = mybir.dt.float32r

N_CORES = 8
B = 16
C = 200
H = W = 56
HP = WP = 58                 # padded spatial
NPIX = H * W                 # 3136
NPAD = HP * WP               # 3364
MARGIN = 64                  # front margin of the padded flat buffer
PADLEN = MARGIN + NPAD + 60  # 3488 per-channel flat length
K_TOP = 313
BN_EPS = 1e-5
IMGS = B // N_CORES          # images per core
CG = 2                       # channel groups (ci and co), 100 each
GC = C // CG                 # 100
QT = 7                       # conv q-tiles, 8 rows each
QROWS = 8
QN = QROWS * WP              # 464 columns per conv matmul
SN = NPIX // QT              # 448 columns per s-matmul tile
PAD_N = 3200                 # kth_largest input size (128 * 25)
NEG = -1.0e30

PRECISION = "f32r"           # "f32r" (fast, ~1e-4 conv err) or "f32" (exact)


def _build_kernel(precision: str):
    nc = bacc.Bacc(None, target_bir_lowering=False)
    mmdt = F32R if precision == "f32r" else F32

    feat_d = nc.dram_tensor("feat", [IMGS, C, PADLEN], F32, kind="ExternalInput")
    ref_d = nc.dram_tensor("ref", [IMGS, C, PADLEN], mmdt, kind="ExternalInput")
    # folded conv weights, laid out [ci_g, ci, (tap, og, co)]
    wl_d = nc.dram_tensor("wl", [CG, GC, 9 * CG * GC], F32, kind="ExternalInput")
    bias2_d = nc.dram_tensor("bias2", [GC, CG], F32, kind="ExternalInput")
    wsc_d = nc.dram_tensor("wsc", [GC, CG], F32, kind="ExternalInput")
    sb_d = nc.dram_tensor("sbias", [1, 1], F32, kind="ExternalInput")
    out_d = nc.dram_tensor("out", [IMGS, 1], F32, kind="ExternalOutput")

    with tile.TileContext(nc) as tc:
        with (
            tc.tile_pool(name="consts", bufs=1) as consts,
            tc.tile_pool(name="stage", bufs=2) as stage,
            tc.tile_pool(name="dpad", bufs=4) as dpad_pool,
            tc.tile_pool(name="hpool", bufs=3) as hpool,
            tc.tile_pool(name="spool", bufs=2) as spool,
            tc.tile_pool(name="small", bufs=2) as small,
            tc.tile_pool(name="cpsum", bufs=4, space="PSUM") as cpsum,
            tc.tile_pool(name="spsum", bufs=2, space="PSUM") as spsum,
        ):
            # ---- constants ----
            wl_f32 = consts.tile([GC, CG, 9 * CG * GC], F32)
            nc.sync.dma_start(out=wl_f32, in_=wl_d[:, :, :].rearrange("g c f -> c g f"))
            bias2 = consts.tile([GC, CG], F32)
            nc.sync.dma_start(out=bias2, in_=bias2_d[:, :])
            wsc_f32 = consts.tile([GC, CG], F32)
            nc.sync.dma_start(out=wsc_f32, in_=wsc_d[:, :])
            sbias = consts.tile([1, 1], F32)
            nc.sync.dma_start(out=sbias, in_=sb_d[:, :])
            if precision == "f32r":
                wl = consts.tile([GC, CG, 9 * CG * GC], F32R)
                nc.vector.tensor_copy(wl, wl_f32)
                wsc = consts.tile([GC, CG], F32R)
                nc.vector.tensor_copy(wsc, wsc_f32)
            else:
                wl, wsc = wl_f32, wsc_f32
            out_sb = consts.tile([1, IMGS], F32)

            for img in range(IMGS):
                # ---- d = ref - feat, in padded layout (in-place on ref) ----
                dpads = []
                for g in range(CG):
                    x_pad = stage.tile([GC, PADLEN], F32, tag="xpad")
                    d_pad = dpad_pool.tile([GC, PADLEN], mmdt, tag="dpad")
                    nc.sync.dma_start(
                        out=x_pad, in_=feat_d[img, g * GC:(g + 1) * GC, :])
                    nc.sync.dma_start(
                        out=d_pad, in_=ref_d[img, g * GC:(g + 1) * GC, :])
                    nc.vector.tensor_tensor(
                        out=d_pad, in0=d_pad, in1=x_pad,
                        op=mybir.AluOpType.subtract,
                    )
                    dpads.append(d_pad)

                # ---- conv 3x3 (+folded BN) + ReLU ----
                hs = []
                for og in range(CG):
                    h_t = hpool.tile([GC, NPIX], mmdt, tag="h")
                    hs.append(h_t)
                    for qt in range(QT):
                        ps = cpsum.tile([GC, QN], F32, tag="cps")
                        i = 0
                        for k in range(9):
                            ky, kx = divmod(k, 3)
                            off = (ky - 1) * WP + (kx - 1)
                            for g in range(CG):
                                base = MARGIN + WP + qt * QN + off
                                nc.tensor.matmul(
                                    ps,
                                    wl[:, g, (k * CG + og) * GC:(k * CG + og + 1) * GC],
                                    dpads[g][:, base:base + QN],
                                    start=(i == 0),
                                    stop=(i == 17),
                                )
                                i += 1
                        # BN+ReLU, keep only interior columns 1..56 per row
                        nc.scalar.activation(
                            out=h_t[:, qt * QROWS * W:(qt + 1) * QROWS * W]
                            .rearrange("p (r c) -> p r c", c=W),
                            in_=ps.rearrange("p (r c) -> p r c", c=WP)[:, :, 1:1 + W],
                            func=mybir.ActivationFunctionType.Relu,
                            bias=bias2[:, og:og + 1],
                            scale=1.0,
                        )

                # ---- s = |conv1x1(h) + sb| ----
                s32 = spool.tile([1, PAD_N], F32, tag="s32")
                nc.vector.memset(s32, NEG)
                for qt in range(QT):
                    sp = spsum.tile([1, SN], F32, tag="sps")
                    for og in range(CG):
                        nc.tensor.matmul(
                            sp,
                            wsc[:, og:og + 1],
                            hs[og][:, qt * SN:(qt + 1) * SN],
                            start=(og == 0),
                            stop=(og == 1),
                        )
                    nc.scalar.activation(
                        out=s32[:, qt * SN:(qt + 1) * SN],
                        in_=sp,
                        func=mybir.ActivationFunctionType.Abs,
                        bias=sbias,
                        scale=1.0,
                    )

                # ---- exact mean of top-313 ----
                s128 = small.tile([128, PAD_N // 128], F32, tag="s128")
                nc.sync.dma_start(out=s128, in_=s32)
                kth = small.tile([1, 2], F32, tag="kth")
                nc.gpsimd.kth_largest(
                    kth, s128, n_per_lane=PAD_N // 128, k=K_TOP,
                    quantile=1.0 - (K_TOP - 1.5) / (NPIX - 1),
                )
                t_bc = small.tile([128, 1], F32, tag="tbc")
                nc.gpsimd.partition_broadcast(t_bc, kth[0:1, 1:2])
                mask = small.tile([128, PAD_N // 128], F32, tag="mask")
                cs = small.tile([128, 2], F32, tag="cs")
                nc.vector.tensor_scalar(
                    out=mask, in0=s128, scalar1=t_bc, scalar2=None,
                    op0=mybir.AluOpType.is_gt,
                )
                nc.vector.tensor_reduce(
                    out=cs[:, 0:1], in_=mask, axis=mybir.AxisListType.X,
                    op=mybir.AluOpType.add,
                )
                masked = small.tile([128, PAD_N // 128], F32, tag="masked")
                nc.vector.tensor_tensor(
                    out=masked, in0=mask, in1=s128, op=mybir.AluOpType.mult
                )
                nc.vector.tensor_reduce(
                    out=cs[:, 1:2], in_=masked, axis=mybir.AxisListType.X,
                    op=mybir.AluOpType.add,
                )
                cs_red = small.tile([128, 2], F32, tag="csred")
                nc.gpsimd.partition_all_reduce(
                    cs_red, cs, channels=128, reduce_op=bass_isa.ReduceOp.add
                )
                tmp = small.tile([1, 1], F32, tag="tmp")
                # tmp = K_TOP - cnt
                nc.vector.tensor_scalar(
                    out=tmp, in0=cs_red[0:1, 0:1], scalar1=-1.0,
                    scalar2=float(K_TOP), op0=mybir.AluOpType.mult,
                    op1=mybir.AluOpType.add,
                )
                nc.vector.tensor_tensor(
                    out=tmp, in0=tmp, in1=kth[0:1, 1:2], op=mybir.AluOpType.mult
                )
                nc.vector.tensor_tensor(
                    out=tmp, in0=tmp, in1=cs_red[0:1, 1:2], op=mybir.AluOpType.add
                )
                nc.vector.tensor_scalar(
                    out=out_sb[:, img:img + 1], in0=tmp, scalar1=1.0 / K_TOP,
                    scalar2=None, op0=mybir.AluOpType.mult,
                )

            nc.sync.dma_start(out=out_d[:, :], in_=out_sb)

    nc.compile()
    return nc


_KERNEL_CACHE = {}


def _get_kernel(precision):
    if precision not in _KERNEL_CACHE:
        _KERNEL_CACHE[precision] = _build_kernel(precision)
    return _KERNEL_CACHE[precision]


def _pad_images(a):
    """[n, C, 56, 56] -> flat padded [n, C, PADLEN] with zero ring/margins."""
    n = a.shape[0]
    out = np.zeros((n, C, PADLEN), np.float32)
    v = out[:, :, MARGIN:MARGIN + NPAD].reshape(n, C, HP, WP)
    v[:, :, 1:1 + H, 1:1 + W] = a
    return out


def _prepare_weights(c_w, c_b, bn_g, bn_b, bn_m, bn_v, score_w, score_b):
    scale = (bn_g / np.sqrt(bn_v + BN_EPS)).astype(np.float32)       # [co]
    wf = (c_w * scale[:, None, None, None]).astype(np.float32)       # [co,ci,3,3]
    bias2 = (scale * (c_b - bn_m) + bn_b).astype(np.float32)         # [co]

    # wl[g, ci, (k*CG+og)*GC + co] = wf[og*GC+co, g*GC+ci, ky, kx]
    w = wf.reshape(CG, GC, C, 3, 3)                  # [og, co, ci, ky, kx]
    w = w.transpose(2, 3, 4, 0, 1)                   # [ci, ky, kx, og, co]
    w = np.ascontiguousarray(w).reshape(CG, GC, 9 * CG * GC)
    wl = np.ascontiguousarray(w, dtype=np.float32)

    bias2_t = np.ascontiguousarray(bias2.reshape(CG, GC).T)          # [GC, og]
    wsc = np.ascontiguousarray(
        score_w.reshape(C).reshape(CG, GC).T.astype(np.float32))     # [GC, og]
    sb = np.array([[np.float32(np.asarray(score_b).reshape(-1)[0])]], np.float32)
    return wl, bias2_t, wsc, sb


def kernel(feature, ref_feature, c1_w, c1_b, c2_w, c2_b, fc1_w, fc1_b,
           fc2_w, fc2_b, comp_conv_w, comp_conv_b, bn_gamma, bn_beta,
           bn_mean, bn_var, score_w, score_b, _trace=False, _precision=None):
    feature = np.asarray(feature, np.float32)
    ref_feature = np.asarray(ref_feature, np.float32)
    wl, bias2, wsc, sb = _prepare_weights(
        np.asarray(comp_conv_w, np.float32), np.asarray(comp_conv_b, np.float32),
        np.asarray(bn_gamma, np.float32), np.asarray(bn_beta, np.float32),
        np.asarray(bn_mean, np.float32), np.asarray(bn_var, np.float32),
        np.asarray(score_w, np.float32), np.asarray(score_b, np.float32))

    feat_pad = _pad_images(feature)
    ref_pad = _pad_images(ref_feature)

    precision = _precision or PRECISION
    nc = _get_kernel(precision)
    in_maps = []
    for r in range(N_CORES):
        sl = slice(r * IMGS, (r + 1) * IMGS)
        in_maps.append(dict(
            feat=np.ascontiguousarray(feat_pad[sl]),
            ref=np.ascontiguousarray(ref_pad[sl]),
            wl=wl, bias2=bias2, wsc=wsc, sbias=sb,
        ))
    res = run_bass_kernel_spmd(
        nc, in_maps, core_ids=list(range(N_CORES)), trace=_trace
    )
    out = np.concatenate([res.results[r]["out"] for r in range(N_CORES)], axis=0)
    if _trace:
        kernel.last_exec_time_ns = res.exec_time_ns
        kernel.last_results = res
    return out.astype(np.float32)


# revision 11
# speedup vs baseline: 3.2233x; 3.2233x over previous
"""Trainium2 Bass kernel for nn_DRA_40072044872030.

Key mathematical identity: in the reference, `_attention_module` applies
softmax over an axis of size 1, which is identically 1.0, so the module is
an exact identity map (wp = p * 1.0). The network therefore reduces to
`_composite_head(feature, ref_feature, ...)`:

    d = ref_feature - feature                         [B, 200, 56, 56]
    h = relu(BN(conv3x3(d, W) + cb))                  [B, 200, 56, 56]
    s = |conv1x1(h, w_s) + sb|                        [B, 56*56]
    out[b] = mean(top_313(s[b]))                      [B, 1]

Device implementation (8 NeuronCores, batch-sharded 2 images/core):
  - BN folded into conv weights/bias on host (weight preprocessing).
  - Images shipped in a zero-padded flat layout [margin | 58*58 | margin]
    per channel so the 3x3 conv becomes 9 shifted contiguous matmuls
    accumulated in PSUM; contraction ci -> 2x100 groups, outputs co ->
    2x100 groups.  d = ref - feat computed on device (in-place DVE sub).
  - float32r (full-rate fp32 matmul mode, ~12-bit mantissa) by default;
    exact fp32 matmul mode via PRECISION = "f32" (4x slower PE).
  - Exact top-k mean via GPSIMD kth_largest (exact 313th-largest value t),
    then mean = (sum(s where s > t) + (313 - count(s > t)) * t) / 313.
"""

import sys

if "/opt/trn_rl_repo" not in sys.path:
    sys.path.insert(0, "/opt/trn_rl_repo")

import numpy as np

import concourse.bass as bass
import concourse.tile as tile
from concourse import bacc, bass_isa, mybir
from concourse.bass_utils import run_bass_kernel_spmd

F32 = mybir.dt.float32
F32R = mybir.dt.float32r

N_CORES = 8
B = 16
C = 200
H = W = 56
HP = WP = 58                 # padded spatial
NPIX = H * W                 # 3136
NPAD = HP * WP               # 3364
MARGIN = 64                  # front margin of the padded flat buffer
PADLEN = MARGIN + NPAD + 60  # 3488 per-channel flat length
K_TOP = 313
BN_EPS = 1e-5
IMGS = B // N_CORES          # images per core
CG = 2                       # channel groups (ci and co), 100 each
GC = C // CG                 # 100
QT = 7                       # conv q-tiles, 8 rows each
QROWS = 8
QN = QROWS * WP              # 464 columns per conv matmul
SN = NPIX // QT              # 448 columns per s-matmul tile
PAD_N = 3200                 # kth_largest input size (128 * 25)
NEG = -1.0e30

PRECISION = "f32r"           # "f32r" (fast, ~1e-4 conv err) or "f32" (exact)


def _build_kernel(precision: str):
    nc = bacc.Bacc(None, target_bir_lowering=False)
    mmdt = F32R if precision == "f32r" else F32

    feat_d = nc.dram_tensor("feat", [IMGS, C, PADLEN], F32, kind="ExternalInput")
    ref_d = nc.dram_tensor("ref", [IMGS, C, PADLEN], mmdt, kind="ExternalInput")
    # folded conv weights, laid out [ci_g, ci, (tap, og, co)]
    wl_d = nc.dram_tensor("wl", [CG, GC, 9 * CG * GC], F32, kind="ExternalInput")
    bias2_d = nc.dram_tensor("bias2", [GC, CG], F32, kind="ExternalInput")
    wsc_d = nc.dram_tensor("wsc", [GC, CG], F32, kind="ExternalInput")
    sb_d = nc.dram_tensor("sbias", [1, 1], F32, kind="ExternalInput")
    # topk consts: col r = (j+1)/128^(r+1) for threshold grids
    tkc_d = nc.dram_tensor("tkc", [128, 3], F32, kind="ExternalInput")
    out_d = nc.dram_tensor("out", [IMGS, 1], F32, kind="ExternalOutput")
    nrounds = 2 if precision == "f32r" else 3

    with tile.TileContext(nc) as tc:
        with (
            tc.tile_pool(name="consts", bufs=1) as consts,
            tc.tile_pool(name="stage", bufs=2) as stage,
            tc.tile_pool(name="dpad", bufs=4) as dpad_pool,
            tc.tile_pool(name="hpool", bufs=3) as hpool,
            tc.tile_pool(name="spool", bufs=2) as spool,
            tc.tile_pool(name="small", bufs=2) as small,
            tc.tile_pool(name="cpsum", bufs=4, space="PSUM") as cpsum,
            tc.tile_pool(name="spsum", bufs=2, space="PSUM") as spsum,
            tc.tile_pool(name="bcast", bufs=1) as bcast,
            tc.tile_pool(name="dram", bufs=2, space="DRAM") as drampool,
        ):
            # ---- constants ----
            wl_f32 = consts.tile([GC, CG, 9 * CG * GC], F32)
            nc.sync.dma_start(out=wl_f32, in_=wl_d[:, :, :].rearrange("g c f -> c g f"))
            bias2 = consts.tile([GC, CG], F32)
            nc.sync.dma_start(out=bias2, in_=bias2_d[:, :])
            wsc_f32 = consts.tile([GC, CG], F32)
            nc.sync.dma_start(out=wsc_f32, in_=wsc_d[:, :])
            sbias = consts.tile([1, 1], F32)
            nc.sync.dma_start(out=sbias, in_=sb_d[:, :])
            tkc = consts.tile([128, 3], F32)
            nc.sync.dma_start(out=tkc, in_=tkc_d[:, :])
            if precision == "f32r":
                wl = consts.tile([GC, CG, 9 * CG * GC], F32R)
                nc.vector.tensor_copy(wl, wl_f32)
                wsc = consts.tile([GC, CG], F32R)
                nc.vector.tensor_copy(wsc, wsc_f32)
            else:
                wl, wsc = wl_f32, wsc_f32
            out_sb = consts.tile([1, IMGS], F32)

            for img in range(IMGS):
                # ---- d = ref - feat, in padded layout (in-place on ref) ----
                dpads = []
                for g in range(CG):
                    x_pad = stage.tile([GC, PADLEN], F32, tag="xpad")
                    d_pad = dpad_pool.tile([GC, PADLEN], mmdt, tag="dpad")
                    nc.sync.dma_start(
                        out=x_pad, in_=feat_d[img, g * GC:(g + 1) * GC, :])
                    nc.sync.dma_start(
                        out=d_pad, in_=ref_d[img, g * GC:(g + 1) * GC, :])
                    nc.vector.tensor_tensor(
                        out=d_pad, in0=d_pad, in1=x_pad,
                        op=mybir.AluOpType.subtract,
                    )
                    dpads.append(d_pad)

                # ---- conv 3x3 (+folded BN) + ReLU ----
                hs = []
                for og in range(CG):
                    h_t = hpool.tile([GC, NPIX], mmdt, tag="h")
                    hs.append(h_t)
                    for qt in range(QT):
                        ps = cpsum.tile([GC, QN], F32, tag="cps")
                        i = 0
                        for k in range(9):
                            ky, kx = divmod(k, 3)
                            off = (ky - 1) * WP + (kx - 1)
                            for g in range(CG):
                                base = MARGIN + WP + qt * QN + off
                                nc.tensor.matmul(
                                    ps,
                                    wl[:, g, (k * CG + og) * GC:(k * CG + og + 1) * GC],
                                    dpads[g][:, base:base + QN],
                                    start=(i == 0),
                                    stop=(i == 17),
                                )
                                i += 1
                        # BN+ReLU, keep only interior columns 1..56 per row
                        nc.scalar.activation(
                            out=h_t[:, qt * QROWS * W:(qt + 1) * QROWS * W]
                            .rearrange("p (r c) -> p r c", c=W),
                            in_=ps.rearrange("p (r c) -> p r c", c=WP)[:, :, 1:1 + W],
                            func=mybir.ActivationFunctionType.Relu,
                            bias=bias2[:, og:og + 1],
                            scale=1.0,
                        )

                # ---- s = |conv1x1(h) + sb| ----
                s32 = spool.tile([1, PAD_N], F32, tag="s32")
                nc.vector.memset(s32, NEG)
                for qt in range(QT):
                    sp = spsum.tile([1, SN], F32, tag="sps")
                    for og in range(CG):
                        nc.tensor.matmul(
                            sp,
                            wsc[:, og:og + 1],
                            hs[og][:, qt * SN:(qt + 1) * SN],
                            start=(og == 0),
                            stop=(og == 1),
                        )
                    nc.scalar.activation(
                        out=s32[:, qt * SN:(qt + 1) * SN],
                        in_=sp,
                        func=mybir.ActivationFunctionType.Abs,
                        bias=sbias,
                        scale=1.0,
                    )

                # ---- approximate 313th-largest threshold t (2-3 rounds of
                # 128-candidate counting; error in t is second-order in the
                # final mean), then exact count+sum against t ----
                s128 = small.tile([128, PAD_N // 128], F32, tag="s128")
                nc.sync.dma_start(out=s128, in_=s32)

                sdram = drampool.tile([NPIX], F32)
                nc.sync.dma_start(out=sdram, in_=s32[0:1, 0:NPIX])
                s_b = bcast.tile([128, NPIX], F32, tag="sb")
                bc_ap = bass.AP(
                    tensor=sdram.tensor, offset=sdram.offset,
                    ap=[[0, 128]] + list(sdram.ap),
                )
                nc.sync.dma_start(out=s_b, in_=bc_ap)

                # m = max(s), replicated on all partitions
                m_col = small.tile([128, 1], F32, tag="mcol")
                nc.vector.tensor_reduce(
                    out=m_col, in_=s_b, axis=mybir.AxisListType.X,
                    op=mybir.AluOpType.max,
                )
                mask = bcast.tile([128, NPIX], F32, tag="mask")
                cnt = small.tile([128, 1], F32, tag="cnt")
                g = small.tile([128, 1], F32, tag="g")
                sg = small.tile([128, 1], F32, tag="sg")
                tfin = small.tile([128, 1], F32, tag="tfin")
                tcand = small.tile([128, 1], F32, tag="tcand")
                u = small.tile([128, 1], F32, tag="u")
                nc.vector.memset(tfin, 0.0)
                for r in range(nrounds):
                    # candidates: tcand_j = tfin + m * (j+1)/128^(r+1)
                    nc.vector.tensor_tensor(
                        out=u, in0=m_col, in1=tkc[:, r:r + 1],
                        op=mybir.AluOpType.mult,
                    )
                    nc.vector.tensor_tensor(
                        out=tcand, in0=u, in1=tfin, op=mybir.AluOpType.add
                    )
                    nc.vector.tensor_scalar(
                        out=mask, in0=s_b, scalar1=tcand, scalar2=None,
                        op0=mybir.AluOpType.is_gt,
                    )
                    nc.vector.tensor_reduce(
                        out=cnt, in_=mask, axis=mybir.AxisListType.X,
                        op=mybir.AluOpType.add,
                    )
                    nc.vector.tensor_scalar(
                        out=g, in0=cnt, scalar1=float(K_TOP), scalar2=None,
                        op0=mybir.AluOpType.is_ge,
                    )
                    nc.gpsimd.partition_all_reduce(
                        sg, g, channels=128, reduce_op=bass_isa.ReduceOp.add
                    )
                    # tfin += m * sg / 128^(r+1)
                    nc.vector.tensor_tensor(
                        out=u, in0=m_col, in1=sg, op=mybir.AluOpType.mult
                    )
                    nc.vector.scalar_tensor_tensor(
                        out=tfin, in0=u, scalar=1.0 / (128.0 ** (r + 1)),
                        in1=tfin, op0=mybir.AluOpType.mult,
                        op1=mybir.AluOpType.add,
                    )

                # exact count & masked sum of s against tfin
                mask25 = small.tile([128, PAD_N // 128], F32, tag="mask25")
                cs = small.tile([128, 2], F32, tag="cs")
                nc.vector.tensor_scalar(
                    out=mask25, in0=s128, scalar1=tfin, scalar2=None,
                    op0=mybir.AluOpType.is_gt,
                )
                nc.vector.tensor_reduce(
                    out=cs[:, 0:1], in_=mask25, axis=mybir.AxisListType.X,
                    op=mybir.AluOpType.add,
                )
                masked = small.tile([128, PAD_N // 128], F32, tag="masked")
                nc.vector.tensor_tensor(
                    out=masked, in0=mask25, in1=s128, op=mybir.AluOpType.mult
                )
                nc.vector.tensor_reduce(
                    out=cs[:, 1:2], in_=masked, axis=mybir.AxisListType.X,
                    op=mybir.AluOpType.add,
                )
                cs_red = small.tile([128, 2], F32, tag="csred")
                nc.gpsimd.partition_all_reduce(
                    cs_red, cs, channels=128, reduce_op=bass_isa.ReduceOp.add
                )
                tmp = small.tile([1, 1], F32, tag="tmp")
                # tmp = K_TOP - cnt
                nc.vector.tensor_scalar(
                    out=tmp, in0=cs_red[0:1, 0:1], scalar1=-1.0,
                    scalar2=float(K_TOP), op0=mybir.AluOpType.mult,
                    op1=mybir.AluOpType.add,
                )
                nc.vector.tensor_tensor(
                    out=tmp, in0=tmp, in1=tfin[0:1, 0:1], op=mybir.AluOpType.mult
                )
                nc.vector.tensor_tensor(
                    out=tmp, in0=tmp, in1=cs_red[0:1, 1:2], op=mybir.AluOpType.add
                )
                nc.vector.tensor_scalar(
                    out=out_sb[:, img:img + 1], in0=tmp, scalar1=1.0 / K_TOP,
                    scalar2=None, op0=mybir.AluOpType.mult,
                )

            nc.sync.dma_start(out=out_d[:, :], in_=out_sb)

    nc.compile()
    return nc


_KERNEL_CACHE = {}


def _get_kernel(precision):
    if precision not in _KERNEL_CACHE:
        _KERNEL_CACHE[precision] = _build_kernel(precision)
    return _KERNEL_CACHE[precision]


def _pad_images(a):
    """[n, C, 56, 56] -> flat padded [n, C, PADLEN] with zero ring/margins."""
    n = a.shape[0]
    out = np.zeros((n, C, PADLEN), np.float32)
    v = out[:, :, MARGIN:MARGIN + NPAD].reshape(n, C, HP, WP)
    v[:, :, 1:1 + H, 1:1 + W] = a
    return out


def _prepare_weights(c_w, c_b, bn_g, bn_b, bn_m, bn_v, score_w, score_b):
    scale = (bn_g / np.sqrt(bn_v + BN_EPS)).astype(np.float32)       # [co]
    wf = (c_w * scale[:, None, None, None]).astype(np.float32)       # [co,ci,3,3]
    bias2 = (scale * (c_b - bn_m) + bn_b).astype(np.float32)         # [co]

    # wl[g, ci, (k*CG+og)*GC + co] = wf[og*GC+co, g*GC+ci, ky, kx]
    w = wf.reshape(CG, GC, C, 3, 3)                  # [og, co, ci, ky, kx]
    w = w.transpose(2, 3, 4, 0, 1)                   # [ci, ky, kx, og, co]
    w = np.ascontiguousarray(w).reshape(CG, GC, 9 * CG * GC)
    wl = np.ascontiguousarray(w, dtype=np.float32)

    bias2_t = np.ascontiguousarray(bias2.reshape(CG, GC).T)          # [GC, og]
    wsc = np.ascontiguousarray(
        score_w.reshape(C).reshape(CG, GC).T.astype(np.float32))     # [GC, og]
    sb = np.array([[np.float32(np.asarray(score_b).reshape(-1)[0])]], np.float32)
    return wl, bias2_t, wsc, sb


def kernel(feature, ref_feature, c1_w, c1_b, c2_w, c2_b, fc1_w, fc1_b,
           fc2_w, fc2_b, comp_conv_w, comp_conv_b, bn_gamma, bn_beta,
           bn_mean, bn_var, score_w, score_b, _trace=False, _precision=None):
    feature = np.asarray(feature, np.float32)
    ref_feature = np.asarray(ref_feature, np.float32)
    wl, bias2, wsc, sb = _prepare_weights(
        np.asarray(comp_conv_w, np.float32), np.asarray(comp_conv_b, np.float32),
        np.asarray(bn_gamma, np.float32), np.asarray(bn_beta, np.float32),
        np.asarray(bn_mean, np.float32), np.asarray(bn_var, np.float32),
        np.asarray(score_w, np.float32), np.asarray(score_b, np.float32))

    feat_pad = _pad_images(feature)
    ref_pad = _pad_images(ref_feature)
    j1 = np.arange(1, 129, dtype=np.float32)[:, None]
    tkc = np.concatenate([j1 / 128.0 ** (r + 1) for r in range(3)], axis=1)
    tkc = np.ascontiguousarray(tkc, np.float32)

    precision = _precision or PRECISION
    nc = _get_kernel(precision)
    in_maps = []
    for r in range(N_CORES):
        sl = slice(r * IMGS, (r + 1) * IMGS)
        in_maps.append(dict(
            feat=np.ascontiguousarray(feat_pad[sl]),
            ref=np.ascontiguousarray(ref_pad[sl]),
            wl=wl, bias2=bias2, wsc=wsc, sbias=sb, tkc=tkc,
        ))
    res = run_bass_kernel_spmd(
        nc, in_maps, core_ids=list(range(N_CORES)), trace=_trace
    )
    out = np.concatenate([res.results[r]["out"] for r in range(N_CORES)], axis=0)
    if _trace:
        kernel.last_exec_time_ns = res.exec_time_ns
        kernel.last_results = res
    return out.astype(np.float32)
